# revision 7
# baseline (speedup 1.0000x reference)
"""Bass/Trainium2 kernel for the AffineTransformLayer (spatial transformer,
bilinear sampling) problem.

Contract: kernel(X, theta) takes FULL inputs
  X [16, 256, 256, 64] fp32, theta [16, 6] fp32
and returns the FULL output [16, 256, 256, 64] fp32, computing the same
bilinear-sampled affine warp as the reference (including its trunc-cast and
clip edge semantics), data-parallel over 8 NeuronCores (2 samples per core).

Per-core design:
  - Gather primitive: nc.gpsimd.dma_gather (InstDMAGatherAnt, non-transpose).
    int16 indices address 32768 units of 512 B stride over one sample; each
    gathered chunk is 768 B = 3 consecutive pixels (64ch fp32). A 3-pixel
    chunk anchored at even pixel 2*hg covers the bilinear x-pair (xg, xg+1)
    for either parity, and max index 255*128+127 = 32767 fits int16.
  - Per output pixel two chunks are gathered (rows y0 and y1); a weighted sum
    with 6 per-pixel weights (3 slots x 2 rows; zero weights on unused slots
    and clipped/collapsed taps) reproduces the reference fp32 arithmetic.
  - The SWDGE descriptor ring caps one gather at ~128 ring entries per lane,
    so each gather moves 1024 indices; 4 gathers run on SWDGE queues 0-3.
  - Index tiles live in the gather ucode's 16-partition-wrapped layout and
    are computed directly in that layout from iota-based coordinate tiles.
  - Weight tiles are computed in the p-major dest-slot layout.
"""

import numpy as np
from contextlib import ExitStack

import concourse.bass as bass
import concourse.tile as tile
from concourse import bacc, mybir
from concourse.bass_utils import run_bass_kernel_spmd

F32 = mybir.dt.float32
I32 = mybir.dt.int32
I16 = mybir.dt.int16
OP = mybir.AluOpType

N_CORES = 8
B_PER_CORE = 2
H = W = 256
C = 64
NPIX_S = H * W                 # pixels per sample (65536)
NPIX = B_PER_CORE * NPIX_S     # pixels per core (131072)
SAMPLE_ELEMS = NPIX_S * C

GN = 1024                      # indices per gather instruction
KPG = GN // 128                # free slots per partition per gather (8)
NQ = 4                         # SWDGE queues / gathers per compute group
GROUP = NQ * GN                # pixels per compute group (4096)
SB = 4                         # compute groups per super-batch
SBPIX = SB * GROUP             # pixels per super-batch (16384)
NSB = NPIX // SBPIX            # super-batches per core (8)
FDW = SBPIX // 128             # p-major free dim per super-batch (128)

_cached = {}


class _F32View:
    """Present an int32 tile through a bitcast-to-f32 AP via [...]."""

    def __init__(self, t):
        self._t = t

    def __getitem__(self, key):
        return self._t[key].bitcast(F32)


def _trunc(nc, pool, x, tag):
    """float trunc-toward-zero of fp32 tile x, matching jnp astype(int32):
    trunc(x) = copysign(floor(|x|), x); floor(|x|) = rint(|x|) - (rint > |x|).
    """
    shp = list(x[:].shape)
    ax = pool.tile(shp, I32, tag=f"{tag}_ax")
    nc.vector.tensor_scalar(ax[:], x[:].bitcast(I32), 0x7FFFFFFF, None,
                            OP.bitwise_and)
    axf = ax[:].bitcast(F32)
    ti = pool.tile(shp, I32, tag=f"{tag}_i")
    nc.vector.tensor_copy(ti[:], axf)           # round-to-nearest-even
    tf = pool.tile(shp, F32, tag=f"{tag}_f")
    nc.vector.tensor_copy(tf[:], ti[:])         # exact
    gt = pool.tile(shp, F32, tag=f"{tag}_gt")
    nc.vector.tensor_tensor(gt[:], tf[:], axf, OP.is_gt)
    fl = pool.tile(shp, F32, tag=f"{tag}_fl")
    nc.vector.tensor_tensor(fl[:], tf[:], gt[:], OP.subtract)
    sgn = pool.tile(shp, I32, tag=f"{tag}_s")
    nc.vector.tensor_scalar(sgn[:], x[:].bitcast(I32), -2147483648, None,
                            OP.bitwise_and)
    out = pool.tile(shp, I32, tag=f"{tag}_o")
    nc.vector.tensor_tensor(out[:], fl[:].bitcast(I32), sgn[:], OP.bitwise_or)
    return _F32View(out)


def _coords(nc, pool, jf, if_, th, s, tag):
    """px, py from fp32 column/row index tiles, replicating reference
    rounding: xs = j*(2/255) - 1; x_pre = t0*xs + t1*ys + t2;
    px = (x_pre + 1) * 128 (the *0.5*256 of the reference is exact)."""
    shp = list(jf[:].shape)
    xsv = pool.tile(shp, F32, tag=f"{tag}_xsv")
    nc.vector.tensor_scalar(xsv[:], jf[:], 2.0 / 255.0, -1.0, OP.mult, OP.add)
    ysv = pool.tile(shp, F32, tag=f"{tag}_ysv")
    nc.vector.tensor_scalar(ysv[:], if_[:], 2.0 / 255.0, -1.0, OP.mult, OP.add)

    out = []
    for r in range(2):
        c0, c1, c2 = 6 * s + 3 * r, 6 * s + 3 * r + 1, 6 * s + 3 * r + 2
        u1 = pool.tile(shp, F32, tag=f"{tag}_u1")
        nc.vector.tensor_scalar(u1[:], xsv[:], th[:, c0:c0 + 1], None, OP.mult)
        u2 = pool.tile(shp, F32, tag=f"{tag}_u2")
        nc.vector.tensor_scalar(u2[:], ysv[:], th[:, c1:c1 + 1], None, OP.mult)
        u3 = pool.tile(shp, F32, tag=f"{tag}_u3")
        nc.vector.tensor_tensor(u3[:], u1[:], u2[:], OP.add)
        u4 = pool.tile(shp, F32, tag=f"{tag}_u4")
        nc.vector.tensor_scalar(u4[:], u3[:], th[:, c2:c2 + 1], None, OP.add)
        p = pool.tile(shp, F32, tag=f"{tag}_p{r}")
        nc.vector.tensor_scalar(p[:], u4[:], 1.0, 128.0, OP.add, OP.mult)
        out.append(p)
    return out


def _clips(nc, pool, v0f, tag):
    """c0=clip(v0), c1=clip(v0+1) from float trunc tile view v0f."""
    shp = list(v0f[:].shape)
    c0 = pool.tile(shp, F32, tag=f"{tag}_c0")
    nc.vector.tensor_scalar(c0[:], v0f[:], 0.0, 255.0, OP.max, OP.min)
    c1a = pool.tile(shp, F32, tag=f"{tag}_c1a")
    nc.vector.tensor_scalar(c1a[:], v0f[:], 1.0, 0.0, OP.add, OP.max)
    c1 = pool.tile(shp, F32, tag=f"{tag}_c1")
    nc.vector.tensor_scalar(c1[:], c1a[:], 255.0, None, OP.min)
    return c0, c1


def _hg(nc, pool, c0, tag):
    """xg = min(c0, 254), hg = floor(xg/2) as float, both exact."""
    shp = list(c0[:].shape)
    xg = pool.tile(shp, F32, tag=f"{tag}_xg")
    nc.vector.tensor_scalar(xg[:], c0[:], 254.0, None, OP.min)
    xgi = pool.tile(shp, I32, tag=f"{tag}_xgi")
    nc.vector.tensor_copy(xgi[:], xg[:])
    hgi = pool.tile(shp, I32, tag=f"{tag}_hgi")
    nc.vector.tensor_scalar(hgi[:], xgi[:], 1, None, OP.arith_shift_right)
    hgf = pool.tile(shp, F32, tag=f"{tag}_hgf")
    nc.vector.tensor_copy(hgf[:], hgi[:])
    return xg, hgf


def build():
    nc = bacc.Bacc(
        "TRN2",
        target_bir_lowering=False,
        debug=False,
        enable_asserts=False,
        num_devices=N_CORES,
        num_swdge_queues=NQ,
    )
    xp = nc.dram_tensor("xp", [B_PER_CORE * SAMPLE_ELEMS + C], F32,
                        kind="ExternalInput")
    th_in = nc.dram_tensor("th", [B_PER_CORE, 6], F32, kind="ExternalInput").ap()
    out_d = nc.dram_tensor("out", [NPIX, C], F32, kind="ExternalOutput").ap()
    th_scratch = nc.dram_tensor("th_scratch", [B_PER_CORE, 6], F32).ap()

    src_aps = [
        bass.AP(xp, b * SAMPLE_ELEMS, [[128, 32768], [1, 192]])
        for b in range(B_PER_CORE)
    ]

    with tile.TileContext(nc) as tc, ExitStack() as ctx:
        pers = ctx.enter_context(tc.tile_pool(name="pers", bufs=1))
        wp = ctx.enter_context(tc.tile_pool(name="wp", bufs=1))
        gp = ctx.enter_context(tc.tile_pool(name="gp", bufs=2))
        cp = ctx.enter_context(tc.tile_pool(name="cp", bufs=2))

        # ---- theta -> [128, 12] broadcast tile ----
        th_sb = pers.tile([B_PER_CORE, 6], F32)
        nc.sync.dma_start(th_sb[:], th_in[:])
        nc.sync.dma_start(th_scratch[:], th_sb[:])
        th = pers.tile([128, 12], F32)
        th_bc_src = bass.AP(th_scratch.tensor, 0, [[0, 128], [1, 12]])
        nc.sync.dma_start(th[:], th_bc_src)

        # ---- per-partition constants ----
        pidx = pers.tile([128, 1], I32)
        nc.gpsimd.iota(pidx[:], pattern=[[0, 1]], base=0, channel_multiplier=1)
        # wrapped-layout base: (p // 32) * GN + (p % 16) * KPG
        p16 = pers.tile([128, 1], I32)
        nc.vector.tensor_scalar(p16[:], pidx[:], 15, None, OP.bitwise_and)
        p16s = pers.tile([128, 1], I32)
        nc.vector.tensor_scalar(p16s[:], p16[:], KPG.bit_length() - 1, None,
                                OP.logical_shift_left)
        p32 = pers.tile([128, 1], I32)
        nc.vector.tensor_scalar(p32[:], pidx[:], 5, None, OP.arith_shift_right)
        p32s = pers.tile([128, 1], I32)
        nc.vector.tensor_scalar(p32s[:], p32[:], GN.bit_length() - 1, None,
                                OP.logical_shift_left)
        padd_w = pers.tile([128, 1], I32)
        nc.vector.tensor_tensor(padd_w[:], p16s[:], p32s[:], OP.add)

        for sb in range(NSB):
            s = sb // (NSB // B_PER_CORE)
            sbbase = sb * SBPIX                 # core-local pixel base
            lbase = sbbase - s * NPIX_S         # sample-local

            # ============ wrapped index pipeline [128, SB*64] ============
            # value(p, col=cs*64 + ci*8 + cq) = lbase + (cs*4 + p//32)*GN
            #     + (16*cq + p%16)*KPG + ci
            WCOL = SB * 64
            wn_i = wp.tile([128, WCOL], I32, tag="wn_i")
            nc.gpsimd.iota(wn_i[:], pattern=[[NQ * GN, SB], [1, 8], [16 * KPG, 8]],
                           base=lbase, channel_multiplier=0)
            wn2 = wp.tile([128, WCOL], I32, tag="wn2")
            nc.vector.tensor_tensor(wn2[:], wn_i[:],
                                    padd_w[:].to_broadcast([128, WCOL]), OP.add)
            wj = wp.tile([128, WCOL], I32, tag="wj")
            nc.vector.tensor_scalar(wj[:], wn2[:], 255, None, OP.bitwise_and)
            wi = wp.tile([128, WCOL], I32, tag="wi")
            nc.vector.tensor_scalar(wi[:], wn2[:], 8, None, OP.arith_shift_right)
            wjf = wp.tile([128, WCOL], F32, tag="wjf")
            nc.vector.tensor_copy(wjf[:], wj[:])
            wif = wp.tile([128, WCOL], F32, tag="wif")
            nc.vector.tensor_copy(wif[:], wi[:])

            wpx, wpy = _coords(nc, wp, wjf, wif, th, s, "wc")
            wx0f = _trunc(nc, wp, wpx, "wtx")
            wy0f = _trunc(nc, wp, wpy, "wty")
            wc0, _wc1 = _clips(nc, wp, wx0f, "wcx")
            wr0, wr1 = _clips(nc, wp, wy0f, "wcy")
            _wxg, whgf = _hg(nc, wp, wc0, "whg")

            idxa_f = wp.tile([128, WCOL], F32, tag="idxa_f")
            nc.vector.scalar_tensor_tensor(idxa_f[:], wr0[:], 128.0, whgf[:],
                                           OP.mult, OP.add)
            idxb_f = wp.tile([128, WCOL], F32, tag="idxb_f")
            nc.vector.scalar_tensor_tensor(idxb_f[:], wr1[:], 128.0, whgf[:],
                                           OP.mult, OP.add)
            idxa = wp.tile([128, WCOL], I16, tag="idxa")
            nc.vector.tensor_copy(idxa[:], idxa_f[:])
            idxb = wp.tile([128, WCOL], I16, tag="idxb")
            nc.vector.tensor_copy(idxb[:], idxb_f[:])

            # ============ p-major weight pipeline [128, FDW] ============
            # pixel(p, col=ch*KPG + k) = lbase + ch*GN + p*KPG + k
            pn_i = wp.tile([128, FDW], I32, tag="pn_i")
            nc.gpsimd.iota(pn_i[:], pattern=[[GN, SBPIX // GN], [1, KPG]],
                           base=lbase, channel_multiplier=KPG)
            pj = wp.tile([128, FDW], I32, tag="pj")
            nc.vector.tensor_scalar(pj[:], pn_i[:], 255, None, OP.bitwise_and)
            pi = wp.tile([128, FDW], I32, tag="pi")
            nc.vector.tensor_scalar(pi[:], pn_i[:], 8, None, OP.arith_shift_right)
            pjf = wp.tile([128, FDW], F32, tag="pjf")
            nc.vector.tensor_copy(pjf[:], pj[:])
            pif = wp.tile([128, FDW], F32, tag="pif")
            nc.vector.tensor_copy(pif[:], pi[:])

            ppx, ppy = _coords(nc, wp, pjf, pif, th, s, "pc")
            px0f = _trunc(nc, wp, ppx, "ptx")
            py0f = _trunc(nc, wp, ppy, "pty")
            c0, c1 = _clips(nc, wp, px0f, "pcx")
            r0, r1 = _clips(nc, wp, py0f, "pcy")
            xg, hgf = _hg(nc, wp, c0, "phg")

            q = wp.tile([128, FDW], F32, tag="q")
            nc.vector.scalar_tensor_tensor(q[:], hgf[:], -2.0, xg[:], OP.mult, OP.add)
            uq = wp.tile([128, FDW], F32, tag="uq")
            nc.vector.tensor_scalar(uq[:], q[:], -1.0, 1.0, OP.mult, OP.add)

            g_ = wp.tile([128, FDW], F32, tag="g_")
            nc.vector.tensor_tensor(g_[:], c1[:], c0[:], OP.subtract)
            wx0a = wp.tile([128, FDW], F32, tag="wx0a")
            nc.vector.tensor_tensor(wx0a[:], c1[:], ppx[:], OP.subtract)
            wx0 = wp.tile([128, FDW], F32, tag="wx0")
            nc.vector.tensor_tensor(wx0[:], wx0a[:], g_[:], OP.mult)
            wx1a = wp.tile([128, FDW], F32, tag="wx1a")
            nc.vector.tensor_tensor(wx1a[:], ppx[:], c0[:], OP.subtract)
            wx1 = wp.tile([128, FDW], F32, tag="wx1")
            nc.vector.tensor_tensor(wx1[:], wx1a[:], g_[:], OP.mult)

            w0 = wp.tile([128, FDW], F32, tag="w0")
            nc.vector.tensor_tensor(w0[:], wx0[:], uq[:], OP.mult)
            w1a = wp.tile([128, FDW], F32, tag="w1a")
            nc.vector.tensor_tensor(w1a[:], wx0[:], q[:], OP.mult)
            w1b = wp.tile([128, FDW], F32, tag="w1b")
            nc.vector.tensor_tensor(w1b[:], wx1[:], uq[:], OP.mult)
            w1 = wp.tile([128, FDW], F32, tag="w1")
            nc.vector.tensor_tensor(w1[:], w1a[:], w1b[:], OP.add)
            w2 = wp.tile([128, FDW], F32, tag="w2")
            nc.vector.tensor_tensor(w2[:], wx1[:], q[:], OP.mult)

            wy0 = wp.tile([128, FDW], F32, tag="wy0")
            nc.vector.tensor_tensor(wy0[:], r1[:], ppy[:], OP.subtract)
            wy1 = wp.tile([128, FDW], F32, tag="wy1")
            nc.vector.tensor_tensor(wy1[:], ppy[:], r0[:], OP.subtract)

            w6 = wp.tile([128, FDW, 6], F32, tag="w6")
            for si, wsl in enumerate((w0, w1, w2)):
                nc.vector.tensor_tensor(w6[:, :, si:si + 1], wsl[:].unsqueeze(-1),
                                        wy0[:].unsqueeze(-1), OP.mult)
                nc.vector.tensor_tensor(w6[:, :, 3 + si:4 + si], wsl[:].unsqueeze(-1),
                                        wy1[:].unsqueeze(-1), OP.mult)

            # ============ gathers + weighted sum per compute group ============
            for cs in range(SB):
                ta = gp.tile([128, NQ * KPG, 192], F32, tag="ta")
                tb = gp.tile([128, NQ * KPG, 192], F32, tag="tb")
                for g in range(NQ):
                    nc.gpsimd.dma_gather(
                        out_ap=ta[:, g * KPG:(g + 1) * KPG, :],
                        in_ap=src_aps[s],
                        idxs_ap=idxa[:, cs * 64:(cs + 1) * 64],
                        num_idxs=GN, num_idxs_reg=GN,
                        elem_size=192, elem_step=128, queue_num=g,
                    )
                    nc.gpsimd.dma_gather(
                        out_ap=tb[:, g * KPG:(g + 1) * KPG, :],
                        in_ap=src_aps[s],
                        idxs_ap=idxb[:, cs * 64:(cs + 1) * 64],
                        num_idxs=GN, num_idxs_reg=GN,
                        elem_size=192, elem_step=128, queue_num=g,
                    )
                wsl = w6[:, cs * (FDW // SB):(cs + 1) * (FDW // SB), :]
                wa_bc = bass.AP(wsl.tensor, wsl.offset,
                                [wsl.ap[0], [6, NQ * KPG], [1, 3], [0, 64]])
                wb_sl = w6[:, cs * (FDW // SB):(cs + 1) * (FDW // SB), 3:6]
                wb_bc = bass.AP(wb_sl.tensor, wb_sl.offset,
                                [wb_sl.ap[0], [6, NQ * KPG], [1, 3], [0, 64]])
                va = ta[:].rearrange("p k (s c) -> p k s c", s=3)
                vb = tb[:].rearrange("p k (s c) -> p k s c", s=3)
                nc.vector.tensor_tensor(va, va, wa_bc, OP.mult)
                nc.vector.tensor_tensor(vb, vb, wb_bc, OP.mult)
                nc.vector.tensor_tensor(ta[:], ta[:], tb[:], OP.add)
                u = cp.tile([128, NQ * KPG, 64], F32, tag="u")
                nc.vector.tensor_tensor(u[:], ta[:, :, 0:64], ta[:, :, 64:128], OP.add)
                ot = cp.tile([128, NQ * KPG, 64], F32, tag="ot")
                nc.vector.tensor_tensor(ot[:], u[:], ta[:, :, 128:192], OP.add)

                # out rows: sbbase + cs*GROUP + g*GN + p*KPG + k
                obase = sbbase + cs * GROUP
                oap = bass.AP(out_d.tensor, obase * C,
                              [[KPG * C, 128], [GN * C, NQ], [1, KPG * C]])
                ovw = ot[:].rearrange("p (g k) c -> p g (k c)", g=NQ)
                nc.sync.dma_start(oap, ovw)

    nc.compile()
    return nc


def _get_nc():
    if "nc" not in _cached:
        _cached["nc"] = build()
    return _cached["nc"]


def kernel(X, theta):
    X = np.ascontiguousarray(X, dtype=np.float32)
    theta = np.ascontiguousarray(theta, dtype=np.float32)
    nc = _get_nc()
    pad = np.zeros(C, dtype=np.float32)
    in_maps = []
    for c in range(N_CORES):
        xs = X[c * B_PER_CORE:(c + 1) * B_PER_CORE].reshape(-1)
        in_maps.append({
            "xp": np.concatenate([xs, pad]),
            "th": theta[c * B_PER_CORE:(c + 1) * B_PER_CORE],
        })
    res = run_bass_kernel_spmd(nc, in_maps, list(range(N_CORES)))
    outs = [res.results[c]["out"].reshape(B_PER_CORE, H, W, C)
            for c in range(N_CORES)]
    return np.concatenate(outs, axis=0)


# revision 9
# speedup vs baseline: 178.9563x; 178.9563x over previous
"""Bass/Trainium2 kernel for the AffineTransformLayer (spatial transformer,
bilinear sampling) problem.

Contract: kernel(X, theta) takes FULL inputs
  X [16, 256, 256, 64] fp32, theta [16, 6] fp32
and returns the FULL output [16, 256, 256, 64] fp32, computing the same
bilinear-sampled affine warp as the reference (including its trunc-cast and
clip edge semantics), data-parallel over 8 NeuronCores (2 samples per core).

Per-core design:
  - Gather primitive: nc.gpsimd.dma_gather (InstDMAGatherAnt, non-transpose).
    int16 indices address 32768 units of 512 B stride over one sample; each
    gathered chunk is 768 B = 3 consecutive pixels (64ch fp32). A 3-pixel
    chunk anchored at even pixel 2*hg covers the bilinear x-pair (xg, xg+1)
    for either parity, and max index 255*128+127 = 32767 fits int16.
  - Per output pixel two chunks are gathered (rows y0 and y1); a weighted sum
    with 6 per-pixel weights (3 slots x 2 rows; zero weights on unused slots
    and clipped/collapsed taps) reproduces the reference fp32 arithmetic.
  - The SWDGE descriptor ring caps one gather at ~128 ring entries per lane,
    so each gather moves 1024 indices; 4 gathers run on SWDGE queues 0-3.
  - Index tiles live in the gather ucode's 16-partition-wrapped layout and
    are computed directly in that layout from iota-based coordinate tiles.
  - Weight tiles are computed in the p-major dest-slot layout.
"""

import numpy as np
from contextlib import ExitStack

import concourse.bass as bass
import concourse.tile as tile
from concourse import bacc, mybir
from concourse.bass_utils import run_bass_kernel_spmd

F32 = mybir.dt.float32
I32 = mybir.dt.int32
I16 = mybir.dt.int16
OP = mybir.AluOpType

N_CORES = 8
B_PER_CORE = 2
H = W = 256
C = 64
NPIX_S = H * W                 # pixels per sample (65536)
NPIX = B_PER_CORE * NPIX_S     # pixels per core (131072)
SAMPLE_ELEMS = NPIX_S * C

GN = 1024                      # indices per gather instruction
KPG = GN // 128                # free slots per partition per gather (8)
NQ = 4                         # SWDGE queues / gathers per compute group
GROUP = NQ * GN                # pixels per compute group (4096)
SB = 4                         # compute groups per super-batch
SBPIX = SB * GROUP             # pixels per super-batch (16384)
NSB = NPIX // SBPIX            # super-batches per core (8)
FDW = SBPIX // 128             # p-major free dim per super-batch (128)

_cached = {}


class _F32View:
    """Present an int32 tile through a bitcast-to-f32 AP via [...]."""

    def __init__(self, t):
        self._t = t

    def __getitem__(self, key):
        return self._t[key].bitcast(F32)


def _trunc(nc, pool, x, tag):
    """float trunc-toward-zero of fp32 tile x, matching jnp astype(int32):
    trunc(x) = copysign(floor(|x|), x); floor(|x|) = rint(|x|) - (rint > |x|).
    """
    shp = list(x[:].shape)
    ax = pool.tile(shp, I32, tag=f"{tag}_ax")
    nc.vector.tensor_scalar(ax[:], x[:].bitcast(I32), 0x7FFFFFFF, None,
                            OP.bitwise_and)
    axf = ax[:].bitcast(F32)
    ti = pool.tile(shp, I32, tag=f"{tag}_i")
    nc.vector.tensor_copy(ti[:], axf)           # round-to-nearest-even
    tf = pool.tile(shp, F32, tag=f"{tag}_f")
    nc.vector.tensor_copy(tf[:], ti[:])         # exact
    gt = pool.tile(shp, F32, tag=f"{tag}_gt")
    nc.vector.tensor_tensor(gt[:], tf[:], axf, OP.is_gt)
    fl = pool.tile(shp, F32, tag=f"{tag}_fl")
    nc.vector.tensor_tensor(fl[:], tf[:], gt[:], OP.subtract)
    sgn = pool.tile(shp, I32, tag=f"{tag}_s")
    nc.vector.tensor_scalar(sgn[:], x[:].bitcast(I32), -2147483648, None,
                            OP.bitwise_and)
    out = pool.tile(shp, I32, tag=f"{tag}_o")
    nc.vector.tensor_tensor(out[:], fl[:].bitcast(I32), sgn[:], OP.bitwise_or)
    return _F32View(out)


def _coords(nc, pool, jf, if_, th, s, tag):
    """px, py from fp32 column/row index tiles, replicating reference
    rounding: xs = j*(2/255) - 1; x_pre = t0*xs + t1*ys + t2;
    px = (x_pre + 1) * 128 (the *0.5*256 of the reference is exact)."""
    shp = list(jf[:].shape)
    xsv = pool.tile(shp, F32, tag=f"{tag}_xsv")
    nc.vector.tensor_scalar(xsv[:], jf[:], 2.0 / 255.0, -1.0, OP.mult, OP.add)
    ysv = pool.tile(shp, F32, tag=f"{tag}_ysv")
    nc.vector.tensor_scalar(ysv[:], if_[:], 2.0 / 255.0, -1.0, OP.mult, OP.add)

    out = []
    for r in range(2):
        c0, c1, c2 = 6 * s + 3 * r, 6 * s + 3 * r + 1, 6 * s + 3 * r + 2
        u1 = pool.tile(shp, F32, tag=f"{tag}_u1")
        nc.vector.tensor_scalar(u1[:], xsv[:], th[:, c0:c0 + 1], None, OP.mult)
        u2 = pool.tile(shp, F32, tag=f"{tag}_u2")
        nc.vector.tensor_scalar(u2[:], ysv[:], th[:, c1:c1 + 1], None, OP.mult)
        u3 = pool.tile(shp, F32, tag=f"{tag}_u3")
        nc.vector.tensor_tensor(u3[:], u1[:], u2[:], OP.add)
        u4 = pool.tile(shp, F32, tag=f"{tag}_u4")
        nc.vector.tensor_scalar(u4[:], u3[:], th[:, c2:c2 + 1], None, OP.add)
        p = pool.tile(shp, F32, tag=f"{tag}_p{r}")
        nc.vector.tensor_scalar(p[:], u4[:], 1.0, 128.0, OP.add, OP.mult)
        out.append(p)
    return out


def _clips(nc, pool, v0f, tag):
    """c0=clip(v0), c1=clip(v0+1) from float trunc tile view v0f."""
    shp = list(v0f[:].shape)
    c0 = pool.tile(shp, F32, tag=f"{tag}_c0")
    nc.vector.tensor_scalar(c0[:], v0f[:], 0.0, 255.0, OP.max, OP.min)
    c1a = pool.tile(shp, F32, tag=f"{tag}_c1a")
    nc.vector.tensor_scalar(c1a[:], v0f[:], 1.0, 0.0, OP.add, OP.max)
    c1 = pool.tile(shp, F32, tag=f"{tag}_c1")
    nc.vector.tensor_scalar(c1[:], c1a[:], 255.0, None, OP.min)
    return c0, c1


def _hg(nc, pool, c0, tag):
    """xg = min(c0, 254), hg = floor(xg/2) as float, both exact."""
    shp = list(c0[:].shape)
    xg = pool.tile(shp, F32, tag=f"{tag}_xg")
    nc.vector.tensor_scalar(xg[:], c0[:], 254.0, None, OP.min)
    xgi = pool.tile(shp, I32, tag=f"{tag}_xgi")
    nc.vector.tensor_copy(xgi[:], xg[:])
    hgi = pool.tile(shp, I32, tag=f"{tag}_hgi")
    nc.vector.tensor_scalar(hgi[:], xgi[:], 1, None, OP.arith_shift_right)
    hgf = pool.tile(shp, F32, tag=f"{tag}_hgf")
    nc.vector.tensor_copy(hgf[:], hgi[:])
    return xg, hgf


def build():
    nc = bacc.Bacc(
        "TRN2",
        target_bir_lowering=False,
        debug=False,
        enable_asserts=False,
        num_devices=N_CORES,
        num_swdge_queues=NQ,
    )
    xp = nc.dram_tensor("xp", [B_PER_CORE * SAMPLE_ELEMS + C], F32,
                        kind="ExternalInput")
    th_in = nc.dram_tensor("th", [B_PER_CORE, 6], F32, kind="ExternalInput").ap()
    out_d = nc.dram_tensor("out", [NPIX, C], F32, kind="ExternalOutput").ap()
    th_scratch = nc.dram_tensor("th_scratch", [B_PER_CORE, 6], F32).ap()

    src_aps = [
        bass.AP(xp, b * SAMPLE_ELEMS, [[128, 32768], [1, 192]])
        for b in range(B_PER_CORE)
    ]

    with tile.TileContext(nc) as tc, ExitStack() as ctx:
        pers = ctx.enter_context(tc.tile_pool(name="pers", bufs=1))
        wp = ctx.enter_context(tc.tile_pool(name="wp", bufs=1))
        ip = ctx.enter_context(tc.tile_pool(name="ip", bufs=2))
        gp = ctx.enter_context(tc.tile_pool(name="gp", bufs=2))
        cp = ctx.enter_context(tc.tile_pool(name="cp", bufs=2))

        # ---- theta -> [128, 12] broadcast tile ----
        th_sb = pers.tile([B_PER_CORE, 6], F32)
        nc.sync.dma_start(th_sb[:], th_in[:])
        nc.sync.dma_start(th_scratch[:], th_sb[:])
        th = pers.tile([128, 12], F32)
        th_bc_src = bass.AP(th_scratch.tensor, 0, [[0, 128], [1, 12]])
        nc.sync.dma_start(th[:], th_bc_src)

        # ---- per-partition constants ----
        pidx = pers.tile([128, 1], I32)
        nc.gpsimd.iota(pidx[:], pattern=[[0, 1]], base=0, channel_multiplier=1)
        # wrapped-layout base: (p // 32) * GN + (p % 16) * KPG
        p16 = pers.tile([128, 1], I32)
        nc.vector.tensor_scalar(p16[:], pidx[:], 15, None, OP.bitwise_and)
        p16s = pers.tile([128, 1], I32)
        nc.vector.tensor_scalar(p16s[:], p16[:], KPG.bit_length() - 1, None,
                                OP.logical_shift_left)
        p32 = pers.tile([128, 1], I32)
        nc.vector.tensor_scalar(p32[:], pidx[:], 5, None, OP.arith_shift_right)
        p32s = pers.tile([128, 1], I32)
        nc.vector.tensor_scalar(p32s[:], p32[:], GN.bit_length() - 1, None,
                                OP.logical_shift_left)
        padd_w = pers.tile([128, 1], I32)
        nc.vector.tensor_tensor(padd_w[:], p16s[:], p32s[:], OP.add)

        for sb in range(NSB):
            s = sb // (NSB // B_PER_CORE)
            sbbase = sb * SBPIX                 # core-local pixel base
            lbase = sbbase - s * NPIX_S         # sample-local

            # ============ wrapped index pipeline [128, SB*64] ============
            # value(p, col=cs*64 + ci*8 + cq) = lbase + (cs*4 + p//32)*GN
            #     + (16*cq + p%16)*KPG + ci
            WCOL = SB * 64
            wn_i = wp.tile([128, WCOL], I32, tag="wn_i")
            nc.gpsimd.iota(wn_i[:], pattern=[[NQ * GN, SB], [1, 8], [16 * KPG, 8]],
                           base=lbase, channel_multiplier=0)
            wn2 = wp.tile([128, WCOL], I32, tag="wn2")
            nc.vector.tensor_tensor(wn2[:], wn_i[:],
                                    padd_w[:].to_broadcast([128, WCOL]), OP.add)
            wj = wp.tile([128, WCOL], I32, tag="wj")
            nc.vector.tensor_scalar(wj[:], wn2[:], 255, None, OP.bitwise_and)
            wi = wp.tile([128, WCOL], I32, tag="wi")
            nc.vector.tensor_scalar(wi[:], wn2[:], 8, None, OP.arith_shift_right)
            wjf = wp.tile([128, WCOL], F32, tag="wjf")
            nc.vector.tensor_copy(wjf[:], wj[:])
            wif = wp.tile([128, WCOL], F32, tag="wif")
            nc.vector.tensor_copy(wif[:], wi[:])

            wpx, wpy = _coords(nc, wp, wjf, wif, th, s, "wc")
            wx0f = _trunc(nc, wp, wpx, "wtx")
            wy0f = _trunc(nc, wp, wpy, "wty")
            wc0, _wc1 = _clips(nc, wp, wx0f, "wcx")
            wr0, wr1 = _clips(nc, wp, wy0f, "wcy")
            _wxg, whgf = _hg(nc, wp, wc0, "whg")

            idxa_f = wp.tile([128, WCOL], F32, tag="idxa_f")
            nc.vector.scalar_tensor_tensor(idxa_f[:], wr0[:], 128.0, whgf[:],
                                           OP.mult, OP.add)
            idxb_f = wp.tile([128, WCOL], F32, tag="idxb_f")
            nc.vector.scalar_tensor_tensor(idxb_f[:], wr1[:], 128.0, whgf[:],
                                           OP.mult, OP.add)
            idxa = ip.tile([128, WCOL], I16, tag="idxa")
            nc.vector.tensor_copy(idxa[:], idxa_f[:])
            idxb = ip.tile([128, WCOL], I16, tag="idxb")
            nc.vector.tensor_copy(idxb[:], idxb_f[:])

            # ============ p-major weight pipeline [128, FDW] ============
            # pixel(p, col=ch*KPG + k) = lbase + ch*GN + p*KPG + k
            pn_i = wp.tile([128, FDW], I32, tag="pn_i")
            nc.gpsimd.iota(pn_i[:], pattern=[[GN, SBPIX // GN], [1, KPG]],
                           base=lbase, channel_multiplier=KPG)
            pj = wp.tile([128, FDW], I32, tag="pj")
            nc.vector.tensor_scalar(pj[:], pn_i[:], 255, None, OP.bitwise_and)
            pi = wp.tile([128, FDW], I32, tag="pi")
            nc.vector.tensor_scalar(pi[:], pn_i[:], 8, None, OP.arith_shift_right)
            pjf = wp.tile([128, FDW], F32, tag="pjf")
            nc.vector.tensor_copy(pjf[:], pj[:])
            pif = wp.tile([128, FDW], F32, tag="pif")
            nc.vector.tensor_copy(pif[:], pi[:])

            ppx, ppy = _coords(nc, wp, pjf, pif, th, s, "pc")
            px0f = _trunc(nc, wp, ppx, "ptx")
            py0f = _trunc(nc, wp, ppy, "pty")
            c0, c1 = _clips(nc, wp, px0f, "pcx")
            r0, r1 = _clips(nc, wp, py0f, "pcy")
            xg, hgf = _hg(nc, wp, c0, "phg")

            q = wp.tile([128, FDW], F32, tag="q")
            nc.vector.scalar_tensor_tensor(q[:], hgf[:], -2.0, xg[:], OP.mult, OP.add)
            uq = wp.tile([128, FDW], F32, tag="uq")
            nc.vector.tensor_scalar(uq[:], q[:], -1.0, 1.0, OP.mult, OP.add)

            g_ = wp.tile([128, FDW], F32, tag="g_")
            nc.vector.tensor_tensor(g_[:], c1[:], c0[:], OP.subtract)
            wx0a = wp.tile([128, FDW], F32, tag="wx0a")
            nc.vector.tensor_tensor(wx0a[:], c1[:], ppx[:], OP.subtract)
            wx0 = wp.tile([128, FDW], F32, tag="wx0")
            nc.vector.tensor_tensor(wx0[:], wx0a[:], g_[:], OP.mult)
            wx1a = wp.tile([128, FDW], F32, tag="wx1a")
            nc.vector.tensor_tensor(wx1a[:], ppx[:], c0[:], OP.subtract)
            wx1 = wp.tile([128, FDW], F32, tag="wx1")
            nc.vector.tensor_tensor(wx1[:], wx1a[:], g_[:], OP.mult)

            w0 = wp.tile([128, FDW], F32, tag="w0")
            nc.vector.tensor_tensor(w0[:], wx0[:], uq[:], OP.mult)
            w1a = wp.tile([128, FDW], F32, tag="w1a")
            nc.vector.tensor_tensor(w1a[:], wx0[:], q[:], OP.mult)
            w1b = wp.tile([128, FDW], F32, tag="w1b")
            nc.vector.tensor_tensor(w1b[:], wx1[:], uq[:], OP.mult)
            w1 = wp.tile([128, FDW], F32, tag="w1")
            nc.vector.tensor_tensor(w1[:], w1a[:], w1b[:], OP.add)
            w2 = wp.tile([128, FDW], F32, tag="w2")
            nc.vector.tensor_tensor(w2[:], wx1[:], q[:], OP.mult)

            wy0 = wp.tile([128, FDW], F32, tag="wy0")
            nc.vector.tensor_tensor(wy0[:], r1[:], ppy[:], OP.subtract)
            wy1 = wp.tile([128, FDW], F32, tag="wy1")
            nc.vector.tensor_tensor(wy1[:], ppy[:], r0[:], OP.subtract)

            w6 = ip.tile([128, FDW, 6], F32, tag="w6")
            for si, wsl in enumerate((w0, w1, w2)):
                nc.vector.tensor_tensor(w6[:, :, si:si + 1], wsl[:].unsqueeze(-1),
                                        wy0[:].unsqueeze(-1), OP.mult)
                nc.vector.tensor_tensor(w6[:, :, 3 + si:4 + si], wsl[:].unsqueeze(-1),
                                        wy1[:].unsqueeze(-1), OP.mult)

            # ============ gathers + weighted sum per compute group ============
            for cs in range(SB):
                ta = gp.tile([128, NQ * KPG, 192], F32, tag="ta")
                tb = gp.tile([128, NQ * KPG, 192], F32, tag="tb")
                for g in range(NQ):
                    nc.gpsimd.dma_gather(
                        out_ap=ta[:, g * KPG:(g + 1) * KPG, :],
                        in_ap=src_aps[s],
                        idxs_ap=idxa[:, cs * 64:(cs + 1) * 64],
                        num_idxs=GN, num_idxs_reg=GN,
                        elem_size=192, elem_step=128, queue_num=g,
                    )
                    nc.gpsimd.dma_gather(
                        out_ap=tb[:, g * KPG:(g + 1) * KPG, :],
                        in_ap=src_aps[s],
                        idxs_ap=idxb[:, cs * 64:(cs + 1) * 64],
                        num_idxs=GN, num_idxs_reg=GN,
                        elem_size=192, elem_step=128, queue_num=g,
                    )
                wsl = w6[:, cs * (FDW // SB):(cs + 1) * (FDW // SB), :]
                wa_bc = bass.AP(wsl.tensor, wsl.offset,
                                [wsl.ap[0], [6, NQ * KPG], [1, 3], [0, 64]])
                wb_sl = w6[:, cs * (FDW // SB):(cs + 1) * (FDW // SB), 3:6]
                wb_bc = bass.AP(wb_sl.tensor, wb_sl.offset,
                                [wb_sl.ap[0], [6, NQ * KPG], [1, 3], [0, 64]])
                va = ta[:].rearrange("p k (s c) -> p k s c", s=3)
                vb = tb[:].rearrange("p k (s c) -> p k s c", s=3)
                nc.vector.tensor_tensor(va, va, wa_bc, OP.mult)
                nc.vector.tensor_tensor(vb, vb, wb_bc, OP.mult)
                nc.vector.tensor_tensor(ta[:], ta[:], tb[:], OP.add)
                u = cp.tile([128, NQ * KPG, 64], F32, tag="u")
                nc.vector.tensor_tensor(u[:], ta[:, :, 0:64], ta[:, :, 64:128], OP.add)
                ot = cp.tile([128, NQ * KPG, 64], F32, tag="ot")
                nc.vector.tensor_tensor(ot[:], u[:], ta[:, :, 128:192], OP.add)

                # out rows: sbbase + cs*GROUP + g*GN + p*KPG + k
                obase = sbbase + cs * GROUP
                oap = bass.AP(out_d.tensor, obase * C,
                              [[KPG * C, 128], [GN * C, NQ], [1, KPG * C]])
                ovw = ot[:].rearrange("p (g k) c -> p g (k c)", g=NQ)
                nc.sync.dma_start(oap, ovw)

    nc.compile()
    return nc


def _get_runner():
    """Build once: the Bass program, the sharded jitted executor, and the
    device-resident zero output buffers. Cached for repeat kernel() calls."""
    if "runner" in _cached:
        return _cached["runner"]

    import jax
    from jax.sharding import Mesh, PartitionSpec, NamedSharding
    from jax.experimental.shard_map import shard_map
    import concourse.bass2jax as bass2jax

    nc = build()
    bass2jax.install_neuronx_cc_hook()

    in_names, out_names, out_avals, zero_outs = [], [], [], []
    pn = nc.partition_id_tensor.name if nc.partition_id_tensor else None
    for alloc in nc.m.functions[0].allocations:
        if not isinstance(alloc, mybir.MemoryLocationSet):
            continue
        name = alloc.memorylocations[0].name
        if alloc.kind == "ExternalInput":
            if name != pn:
                in_names.append(name)
        elif alloc.kind == "ExternalOutput":
            out_names.append(name)
            shape = tuple(alloc.tensor_shape)
            dtype = mybir.dt.np(alloc.dtype)
            out_avals.append(jax.core.ShapedArray(shape, dtype))
            zero_outs.append(np.zeros(shape, dtype))

    def _body(*args):
        ops = list(args)
        if pn is not None:
            ops.append(bass2jax.partition_id_tensor())
        return tuple(bass2jax._bass_exec_p.bind(
            *ops,
            out_avals=tuple(out_avals),
            in_names=tuple(list(in_names) + out_names + ([pn] if pn else [])),
            out_names=tuple(out_names),
            lowering_input_output_aliases=(),
            sim_require_finite=True,
            sim_require_nnan=True,
            nc=nc,
        ))

    devices = jax.devices()[:N_CORES]
    mesh = Mesh(np.asarray(devices), ("core",))
    nin = len(in_names) + len(out_names)
    fn = jax.jit(
        shard_map(_body, mesh=mesh, in_specs=(PartitionSpec("core"),) * nin,
                  out_specs=(PartitionSpec("core"),) * len(out_names),
                  check_rep=False),
        keep_unused=True,
    )
    sh = NamedSharding(mesh, PartitionSpec("core"))
    dz = [jax.device_put(np.zeros((N_CORES * z.shape[0], *z.shape[1:]), z.dtype), sh)
          for z in zero_outs]
    runner = {
        "fn": fn, "dz": dz, "sh": sh, "in_names": in_names,
        "out_idx": out_names.index("out"), "device_put": jax.device_put,
    }
    _cached["runner"] = runner
    return runner


def kernel(X, theta):
    X = np.ascontiguousarray(X, dtype=np.float32)
    theta = np.ascontiguousarray(theta, dtype=np.float32)
    r = _get_runner()
    pad = np.zeros(C, dtype=np.float32)
    xp_all = np.empty((N_CORES, B_PER_CORE * SAMPLE_ELEMS + C), dtype=np.float32)
    for c in range(N_CORES):
        xp_all[c, :B_PER_CORE * SAMPLE_ELEMS] = (
            X[c * B_PER_CORE:(c + 1) * B_PER_CORE].reshape(-1))
        xp_all[c, B_PER_CORE * SAMPLE_ELEMS:] = pad
    per_name = {
        "xp": xp_all.reshape(-1),
        "th": theta.reshape(N_CORES * B_PER_CORE, 6),
    }
    di = [r["device_put"](per_name[nm], r["sh"]) for nm in r["in_names"]]
    out = r["fn"](*di, *r["dz"])
    res = np.asarray(out[r["out_idx"]])
    return res.reshape(N_CORES * B_PER_CORE, H, W, C)
